# revision 34
# baseline (speedup 1.0000x reference)
"""Trainium2 Bass kernel for nn_ConditionalPoolingModule.

Reference computation (per scene s of 64, peds i,j of 64):
    feat[s,i,j]  = [pos_j - pos_i, speed_j]
    emb          = feat @ W_emb + b_emb
    x1[s,i,j]    = relu(bn1(concat(h_j, emb) @ W1 + b1))      # [.., 512]
    x2[s,i,j]    = relu(bn2(x1 @ W2 + b2))                    # [.., 256]
    out[s,i]     = max_j x2[s,i,j]

Algebra (same as the prior version):
  * Layer 1 is separable: bn1-affine(concat(h_j, emb_ij) @ W1 + b1) = A''[j] - B''[i]
    with A''[n] = s1*(X[n] @ W1aug) + (s1*c0 + t1), B''[n] = s1*(pos[n] @ R[:2]),
    X = [h, posx, posy, speed] (67 dims), R = W_emb @ W1[64:80], W1aug = [W1[:64]; R].
  * relu(a-b) = max(a,b) - b, and -B''[i] is j-constant, so
      out[i] = relu( max_j( max(A''[j], B''[i]) @ W2s + t2 ) - B''[i] @ W2s )
    (t2 is folded into the PSUM->SBUF drain bias; B''@W2s is the per-i D term).

Schedule changes vs the 143.5us version (cost-model driven):
  * PSUM->SBUF drains are 1024 wide (2 banks per ACT instruction) and add the
    bn2 shift t2 as a per-partition bias, dropping the separate t2 matmul.
  * All D terms are computed upfront, transposed (lhsT = W2s chunk, rhs = B''
    in fp16, 64 moving rows) and drained to one fp16 SBUF tile; they fill PE
    while DVE builds scene-0 Mx. The epilogue runs in [m-chunk, i]
    orientation: one fp16 DVE subtract (2x), one small fp16 PE transpose, ACT
    relu, DMA out. This removes 40 f32r matmuls + 16 f32 transposes from PE
    and all D copies from ACT's steady state.
  * j-max trees use TT-max halvings all the way down (the 1x grouped reduce
    is slower than three tiny 2x TTs); the final (scene, m) tree runs
    fine-grained per drain block to shorten the pipeline tail.
  * Scene-0 Mx is emitted in i-quarters (first quarter reads B'' directly at
    1x) so layer-2 starts ~4us earlier; input DMAs are spread across SP/ACT/
    Pool sequencers (each dma_start costs ~600ns of issuing-SEQ time).
  * NOTE: TensorTensor/TensorReduce on the Pool engine pass the cost model
    but FAIL walrus codegen ("Instruction engine check failed (Pool)") -- the
    V3 Pool/GPSIMD engine cannot run generic vector ops, so everything
    element-wise lives on DVE (~115us busy) and ACT (~85us).

Measured on 8-core axon trn2: relative error 8.5e-4 vs fp32 reference;
cost-model (TimelineSim) kernel time ~133.3us (baseline 143.5us).
"""
import numpy as np

import concourse.bacc as bacc
import concourse.tile as tile
from concourse import mybir
from concourse.bass_utils import run_bass_kernel_spmd

EPS = 1e-5
S, P = 64, 64
H, E = 64, 16
MID, BOT = 512, 256
KIN = H + 3            # 67: h(64) + posx + posy + speed
NCORES = 8
S_LOC = S // NCORES    # 8 scenes per core
NLOC = S_LOC * P       # 512 peds per core
KC = MID // 128        # 4 mid chunks
MC = BOT // 128        # 2 bot chunks
F32 = mybir.dt.float32
F32R = mybir.dt.float32r
F16 = mybir.dt.float16

_CACHE = {}


def _build_program():
    nc = bacc.Bacc("TRN2", target_bir_lowering=False, debug=False, num_devices=NCORES)

    xaugT = nc.dram_tensor("xaugT", [KIN, NLOC], F32, kind="ExternalInput").ap()
    w1augT = nc.dram_tensor("w1augT", [KIN, MID], F32, kind="ExternalInput").ap()
    w2sb16 = nc.dram_tensor("w2sb16", [MID, BOT], F16, kind="ExternalInput").ap()
    midvec = nc.dram_tensor("midvec", [MID, 2], F32, kind="ExternalInput").ap()
    t2c = nc.dram_tensor("t2c", [128, MC], F32, kind="ExternalInput").ap()
    identh = nc.dram_tensor("identh", [128, 128], F16, kind="ExternalInput").ap()
    out_d = nc.dram_tensor("out", [NLOC, BOT], F32, kind="ExternalOutput").ap()

    with tile.TileContext(nc) as tc, \
         tc.tile_pool(name="const", bufs=1) as cpool, \
         tc.tile_pool(name="ab", bufs=1) as abpool, \
         tc.tile_pool(name="mx", bufs=3) as mxpool, \
         tc.tile_pool(name="y2", bufs=3) as ypool, \
         tc.tile_pool(name="work", bufs=2) as wpool, \
         tc.tile_pool(name="mm", bufs=3, space="PSUM") as mmpool, \
         tc.tile_pool(name="dps", bufs=1, space="PSUM") as dpool, \
         tc.tile_pool(name="tps", bufs=1, space="PSUM") as tpool:

        # ---- load constants (spread across SEQs: each dma_start costs
        # ~600ns of issuing-sequencer time, so one queue would serialize) ----
        xaug_sb = cpool.tile([KIN, NLOC], F32R)
        w1aug_sb = cpool.tile([KIN, MID], F32R)
        idh_sb = cpool.tile([128, 128], F16)
        t2_sb = cpool.tile([128, MC], F32)
        # w1aug chunk 0 first (tiny) so the first phase-0 matmul can start
        # while the rest of the weights stream in
        nc.scalar.dma_start(w1aug_sb[:, 0:128], w1augT[:, 0:128].bitcast(F32R))
        nc.sync.dma_start(xaug_sb[:], xaugT.bitcast(F32R))
        nc.scalar.dma_start(w1aug_sb[:, 128:MID], w1augT[:, 128:MID].bitcast(F32R))
        # posT / w1b are row slices of tensors already in SBUF
        posT_sb = xaug_sb
        w1b_sb = w1aug_sb
        nc.gpsimd.dma_start(idh_sb[:], identh)
        nc.gpsimd.dma_start(t2_sb[:], t2c)

        w2b_sb, mv_sb = [], []
        for k in range(KC):
            wb = cpool.tile([128, BOT], F16, tag=f"w2b{k}")
            (nc.sync if k % 2 else nc.scalar).dma_start(
                wb[:], w2sb16[k * 128:(k + 1) * 128, :])
            w2b_sb.append(wb)
            mv = cpool.tile([128, 2], F32, tag=f"mv{k}")
            nc.gpsimd.dma_start(mv[:], midvec[k * 128:(k + 1) * 128, :])
            mv_sb.append(mv)

        # ---- phase 0: A'' (fp16), B'' (fp16) and dup-B (fp16) ----
        A_bf, B_bf, B_dup = [], [], []
        for k in range(KC):
            ck = slice(k * 128, (k + 1) * 128)
            ps = mmpool.tile([128, 1024], F32, tag="ps")
            nc.tensor.matmul(ps[:, 0:512], lhsT=w1aug_sb[:, ck], rhs=xaug_sb[:],
                             start=True, stop=True)
            nc.tensor.matmul(ps[:, 512:1024], lhsT=w1b_sb[H:H + 2, ck], rhs=posT_sb[H:H + 2, :],
                             start=True, stop=True)
            a_t = abpool.tile([128, NLOC], F16, tag=f"A{k}")
            nc.scalar.activation(
                a_t[:], ps[:, 0:512], mybir.ActivationFunctionType.Identity,
                bias=mv_sb[k][:, 1:2], scale=mv_sb[k][:, 0:1])
            A_bf.append(a_t)
            b_t = abpool.tile([128, NLOC], F16, tag=f"B{k}")
            nc.scalar.activation(
                b_t[:], ps[:, 512:1024], mybir.ActivationFunctionType.Identity,
                bias=0.0, scale=mv_sb[k][:, 0:1])
            B_bf.append(b_t)
            # duplicated fp16 copy: B_dup[c, 2n+q] = B''[c, n], q in {0,1}
            bd = abpool.tile([128, 2 * NLOC], F16, tag=f"Bd{k}")
            nc.scalar.activation(
                bd[:].rearrange("c (n q) -> c n q", q=2),
                ps[:, 512:1024].unsqueeze(2).broadcast_to((128, NLOC, 2)),
                mybir.ActivationFunctionType.Identity,
                bias=0.0, scale=mv_sb[k][:, 0:1])
            B_dup.append(bd)

        # ---- all D terms upfront (PE filler while DVE builds scene-0 Mx):
        # d16[c, (s, m, i)] = (W2s chunk-m)^T @ B''_scene, drained to fp16 ----
        d16 = abpool.tile([128, S_LOC * MC * P], F16, tag="d16")
        for s in range(S_LOC):
            for m in range(MC):
                dp = dpool.tile([128, P], F32, tag="dps", name="dp")
                for k in range(KC):
                    nc.tensor.matmul(dp[:], lhsT=w2b_sb[k][:, m * 128:(m + 1) * 128],
                                     rhs=B_bf[k][:, s * P:(s + 1) * P],
                                     start=(k == 0), stop=(k == KC - 1))
                idx = s * MC + m
                nc.scalar.copy(d16[:, idx * P:(idx + 1) * P], dp[:])

        def epilogue(sp, maxraw_p):
            # epilogue in [m-chunk, i] orientation:
            #   o_T = maxraw - d16 (DVE fp16 at 2x),
            #   fp16 PE transpose, ACT relu (fp16 PSUM -> f32 SBUF), DMA out.
            o_sb = wpool.tile([P, BOT], F32, tag="osb")
            for m in range(MC):
                idx = sp * MC + m
                o_t = wpool.tile([128, P], F16, tag=f"ot{m}", name=f"ot_{m}")
                nc.vector.tensor_tensor(
                    out=o_t[:], in0=maxraw_p[m][:],
                    in1=d16[:, idx * P:(idx + 1) * P],
                    op=mybir.AluOpType.subtract)
                tr = tpool.tile([P, 128], F16, tag="tr")
                nc.tensor.transpose(tr[:], o_t[:], idh_sb[:])
                nc.scalar.activation(o_sb[:, m * 128:(m + 1) * 128], tr[:],
                                     mybir.ActivationFunctionType.Relu)
            nc.sync.dma_start(out_d[sp * P:(sp + 1) * P, :], o_sb[:])

        # ---- per-scene pipeline (epilogue of scene s-1 is emitted inside
        # scene s so no engine stream waits on same-scene late results) ----
        prev = None
        for s in range(S_LOC):
            cs = slice(s * P, (s + 1) * P)
            cs2 = slice(2 * s * P, 2 * (s + 1) * P)

            # Mx[c, i, j] = max(A''[c, j], B''[c, i]) in fp16 at 2x.
            # Layout [c, (i, jw, q)] with j = 2*jw + q; every operand's
            # innermost AP dim is [step 1, 2 elems] so DVE picks 2x_1P.
            # Scene 0 is emitted in i-quarters so the first L2 block-pair
            # can start after ~1/4 of the Mx work (pipeline fill).
            mx = [mxpool.tile([128, P * P], F16, tag=f"mx{k}", name=f"mx_{k}")
                  for k in range(KC)]

            def mx_tt(k, ilo, ihi, eng):
                ni = ihi - ilo
                eng.tensor_tensor(
                    out=mx[k][:, ilo * P:ihi * P]
                        .rearrange("c (i w q) -> c i w q", w=P // 2, q=2),
                    in0=A_bf[k][:, cs].rearrange("c (w q) -> c w q", q=2)
                        .unsqueeze(1).broadcast_to((128, ni, P // 2, 2)),
                    in1=B_dup[k][:, 2 * (s * P + ilo):2 * (s * P + ihi)]
                        .rearrange("c (i q) -> c i q", q=2)
                        .unsqueeze(2).broadcast_to((128, ni, P // 2, 2)),
                    op=mybir.AluOpType.max)

            # Pool helps with chunk 3 for the first scenes (pipeline fill);
            # scene 0 is emitted in i-quarters so the first L2 block-pair
            # can start after ~1/4 of the Mx work.
            if s == 0:
                # quarter 0 reads B'' directly (1x, no dup needed) so the
                # first L2 block-pair isn't gated on the B_dup ACT chain
                for k in range(KC):
                    nc.vector.tensor_tensor(
                        out=mx[k][:, 0:16 * P]
                            .rearrange("c (i w q) -> c i w q", w=P // 2, q=2),
                        in0=A_bf[k][:, cs].rearrange("c (w q) -> c w q", q=2)
                            .unsqueeze(1).broadcast_to((128, 16, P // 2, 2)),
                        in1=B_bf[k][:, s * P:s * P + 16]
                            .unsqueeze(2).unsqueeze(3)
                            .broadcast_to((128, 16, P // 2, 2)),
                        op=mybir.AluOpType.max)
                for quarter in range(1, 4):
                    for k in range(KC):
                        mx_tt(k, quarter * 16, (quarter + 1) * 16, nc.vector)
            else:
                for k in range(KC):
                    eng = nc.vector
                    mx_tt(k, 0, P, eng)

            maxraw = [wpool.tile([128, P], F16, tag=f"mr{m}", name=f"mr_{m}")
                      for m in range(MC)]
            for m in range(MC):
                fine_tail = (s == S_LOC - 1 and m == MC - 1)
                # layer-2 matmuls (fp16); ACT drains 1024-wide with bias t2
                y2 = ypool.tile([128, P * P], F16, tag=f"y2{m}", name=f"y2_{m}")
                y3 = y2[:].rearrange("c (i j) -> c i j", j=P)
                r1 = wpool.tile([128, P * 32], F16, tag="r1", name="r1")
                for bp in range(4):
                    ps_t = mmpool.tile([128, 1024], F32, tag="ps")
                    for half in range(2):
                        bs = slice((2 * bp + half) * 512, (2 * bp + half + 1) * 512)
                        for k in range(KC):
                            nc.tensor.matmul(
                                ps_t[:, half * 512:(half + 1) * 512],
                                lhsT=w2b_sb[k][:, m * 128:(m + 1) * 128],
                                rhs=mx[k][:, bs],
                                start=(k == 0), stop=(k == KC - 1))
                    nc.scalar.activation(
                        y2[:, bp * 1024:(bp + 1) * 1024], ps_t[:],
                        mybir.ActivationFunctionType.Identity,
                        bias=t2_sb[:, m:m + 1], scale=1.0)
                    if fine_tail:
                        # last scene: fully fine-grained trees, interleaved
                        # with the drains, to shorten the pipeline tail
                        isl = slice(bp * 16, (bp + 1) * 16)
                        r1v = r1[:].rearrange("c (i j) -> c i j", j=32)
                        nc.vector.tensor_tensor(
                            out=r1v[:, isl, :],
                            in0=y3[:, isl, 0:32], in1=y3[:, isl, 32:64],
                            op=mybir.AluOpType.max)
                        r2f = wpool.tile([128, P * 16], F16, tag="r2", name="r2f")
                        r2fv = r2f[:].rearrange("c (i j) -> c i j", j=16)
                        nc.vector.tensor_tensor(
                            out=r2fv[:, isl, :],
                            in0=r1v[:, isl, 0:16], in1=r1v[:, isl, 16:32],
                            op=mybir.AluOpType.max)
                        r3f = wpool.tile([128, P * 8], F16, tag="r3", name="r3f")
                        r3fv = r3f[:].rearrange("c (i j) -> c i j", j=8)
                        nc.vector.tensor_tensor(
                            out=r3fv[:, isl, :],
                            in0=r2fv[:, isl, 0:8], in1=r2fv[:, isl, 8:16],
                            op=mybir.AluOpType.max)
                        nc.vector.tensor_reduce(
                            out=maxraw[m][:, isl],
                            in_=r3fv[:, isl, :],
                            axis=mybir.AxisListType.X,
                            op=mybir.AluOpType.max)

                # j-max: TT-max halvings at 2x all the way down (the 1x
                # grouped reduce is slower than three tiny 2x TTs)
                if not fine_tail:
                    nc.vector.tensor_tensor(
                        out=r1[:].rearrange("c (i j) -> c i j", j=32),
                        in0=y3[:, :, 0:32], in1=y3[:, :, 32:64],
                        op=mybir.AluOpType.max)
                    r1v = r1[:].rearrange("c (i j) -> c i j", j=32)
                    r2 = wpool.tile([128, P * 16], F16, tag="r2", name="r2")
                    nc.vector.tensor_tensor(
                        out=r2[:].rearrange("c (i j) -> c i j", j=16),
                        in0=r1v[:, :, 0:16], in1=r1v[:, :, 16:32],
                        op=mybir.AluOpType.max)
                    r2v = r2[:].rearrange("c (i j) -> c i j", j=16)
                    r3 = wpool.tile([128, P * 8], F16, tag="r3", name="r3")
                    nc.vector.tensor_tensor(
                        out=r3[:].rearrange("c (i j) -> c i j", j=8),
                        in0=r2v[:, :, 0:8], in1=r2v[:, :, 8:16],
                        op=mybir.AluOpType.max)
                    r3v = r3[:].rearrange("c (i j) -> c i j", j=8)
                    r4 = wpool.tile([128, P * 4], F16, tag="r4", name="r4")
                    nc.vector.tensor_tensor(
                        out=r4[:].rearrange("c (i j) -> c i j", j=4),
                        in0=r3v[:, :, 0:4], in1=r3v[:, :, 4:8],
                        op=mybir.AluOpType.max)
                    r4v = r4[:].rearrange("c (i j) -> c i j", j=4)
                    r5 = wpool.tile([128, P * 2], F16, tag="r5", name="r5")
                    nc.vector.tensor_tensor(
                        out=r5[:].rearrange("c (i j) -> c i j", j=2),
                        in0=r4v[:, :, 0:2], in1=r4v[:, :, 2:4],
                        op=mybir.AluOpType.max)
                    r5v = r5[:].rearrange("c (i j) -> c i j", j=2)
                    nc.vector.tensor_tensor(
                        out=maxraw[m][:].unsqueeze(2),
                        in0=r5v[:, :, 0:1], in1=r5v[:, :, 1:2],
                        op=mybir.AluOpType.max)

            # previous scene's epilogue (keeps PE/DVE streams off
            # same-scene late results)
            if prev is not None:
                epilogue(*prev)
            prev = (s, maxraw)
        epilogue(*prev)

    nc.compile()
    return nc


def _prep_inputs(inputs):
    h = np.ascontiguousarray(inputs["h_states"], np.float32)
    pos = np.ascontiguousarray(inputs["last_pos"], np.float32)
    spd = np.ascontiguousarray(inputs["speed"], np.float32)
    W_emb = np.asarray(inputs["W_emb"], np.float32)
    b_emb = np.asarray(inputs["b_emb"], np.float32)
    W1 = np.asarray(inputs["W1"], np.float32)
    b1 = np.asarray(inputs["b1"], np.float32)
    g1 = np.asarray(inputs["g1"], np.float32)
    be1 = np.asarray(inputs["be1"], np.float32)
    m1 = np.asarray(inputs["m1"], np.float32)
    v1 = np.asarray(inputs["v1"], np.float32)
    W2 = np.asarray(inputs["W2"], np.float32)
    b2 = np.asarray(inputs["b2"], np.float32)
    g2 = np.asarray(inputs["g2"], np.float32)
    be2 = np.asarray(inputs["be2"], np.float32)
    m2 = np.asarray(inputs["m2"], np.float32)
    v2 = np.asarray(inputs["v2"], np.float32)

    s1 = g1 / np.sqrt(v1 + EPS)
    t1 = be1 - m1 * s1
    s2 = g2 / np.sqrt(v2 + EPS)
    t2 = be2 - m2 * s2 + b2 * s2
    R3 = W_emb @ W1[H:H + E, :]                       # [3, MID]
    W1aug = np.concatenate([W1[:H, :], R3], axis=0)   # [67, MID]
    c0v = b1 + b_emb @ W1[H:H + E, :]                 # [MID]
    ca = s1 * c0v + t1
    W2s = W2 * s2[None, :]                            # [MID, BOT]

    X = np.concatenate([h, pos[:, 0:1], pos[:, 1:2], spd], axis=1)  # [N, 67]

    common = dict(
        w1augT=np.ascontiguousarray(W1aug, np.float32),
        w2sb16=np.ascontiguousarray(W2s.astype(np.float16)),
        midvec=np.ascontiguousarray(np.stack([s1, ca], axis=1), np.float32),
        t2c=np.ascontiguousarray(t2.reshape(MC, 128).T, np.float32),
        identh=np.eye(128, dtype=np.float16),
    )
    in_maps = []
    for c in range(NCORES):
        xc = X[c * NLOC:(c + 1) * NLOC, :]            # [512, 67]
        m = dict(common)
        m["xaugT"] = np.ascontiguousarray(xc.T, np.float32)
        in_maps.append(m)
    return in_maps


def kernel(**inputs):
    if "nc" not in _CACHE:
        _CACHE["nc"] = _build_program()
    nc = _CACHE["nc"]
    in_maps = _prep_inputs(inputs)
    res = run_bass_kernel_spmd(nc, in_maps, list(range(NCORES)))
    out = np.concatenate([res.results[c]["out"] for c in range(NCORES)], axis=0)
    return np.ascontiguousarray(out, np.float32)


# revision 40
# speedup vs baseline: 1.0129x; 1.0129x over previous
"""Trainium2 Bass kernel for nn_ConditionalPoolingModule.

Reference computation (per scene s of 64, peds i,j of 64):
    feat[s,i,j]  = [pos_j - pos_i, speed_j]
    emb          = feat @ W_emb + b_emb
    x1[s,i,j]    = relu(bn1(concat(h_j, emb) @ W1 + b1))      # [.., 512]
    x2[s,i,j]    = relu(bn2(x1 @ W2 + b2))                    # [.., 256]
    out[s,i]     = max_j x2[s,i,j]

Algebra (same as the prior version):
  * Layer 1 is separable: bn1-affine(concat(h_j, emb_ij) @ W1 + b1) = A''[j] - B''[i]
    with A''[n] = s1*(X[n] @ W1aug) + (s1*c0 + t1), B''[n] = s1*(pos[n] @ R[:2]),
    X = [h, posx, posy, speed] (67 dims), R = W_emb @ W1[64:80], W1aug = [W1[:64]; R].
  * relu(a-b) = max(a,b) - b, and -B''[i] is j-constant, so
      out[i] = relu( max_j( max(A''[j], B''[i]) @ W2s + t2 ) - B''[i] @ W2s )
    (t2 is folded into the PSUM->SBUF drain bias; B''@W2s is the per-i D term).

Schedule changes vs the 143.5us version (cost-model driven):
  * PSUM->SBUF drains are 1024 wide (2 banks per ACT instruction) and add the
    bn2 shift t2 as a per-partition bias, dropping the separate t2 matmul.
  * All D terms are computed upfront, transposed (lhsT = W2s chunk, rhs = B''
    in fp16, 64 moving rows) and drained to one fp16 SBUF tile; they fill PE
    while DVE builds scene-0 Mx. The epilogue runs in [m-chunk, i]
    orientation: one fp16 DVE subtract (2x), one small fp16 PE transpose, ACT
    relu, DMA out. This removes 40 f32r matmuls + 16 f32 transposes from PE
    and all D copies from ACT's steady state.
  * j-max trees use TT-max halvings all the way down (the 1x grouped reduce
    is slower than three tiny 2x TTs); the final (scene, m) tree runs
    fine-grained per drain block to shorten the pipeline tail.
  * Scene-0 Mx is emitted in i-quarters (first quarter reads B'' directly at
    1x) so layer-2 starts ~4us earlier; input DMAs are spread across SP/ACT/
    Pool sequencers (each dma_start costs ~600ns of issuing-SEQ time).
  * NOTE: TensorTensor/TensorReduce on the Pool engine pass the cost model
    but FAIL walrus codegen ("Instruction engine check failed (Pool)") -- the
    V3 Pool/GPSIMD engine cannot run generic vector ops, so everything
    element-wise lives on DVE (~115us busy) and ACT (~85us).

Measured on 8-core axon trn2: relative error 8.5e-4 vs fp32 reference;
cost-model (TimelineSim) kernel time ~133.3us (baseline 143.5us).
"""
import numpy as np

import concourse.bacc as bacc
import concourse.tile as tile
from concourse import mybir
from concourse.bass_utils import run_bass_kernel_spmd

EPS = 1e-5
S, P = 64, 64
H, E = 64, 16
MID, BOT = 512, 256
KIN = H + 3            # 67: h(64) + posx + posy + speed
NCORES = 8
S_LOC = S // NCORES    # 8 scenes per core
NLOC = S_LOC * P       # 512 peds per core
KC = MID // 128        # 4 mid chunks
MC = BOT // 128        # 2 bot chunks
F32 = mybir.dt.float32
F32R = mybir.dt.float32r
F16 = mybir.dt.float16

_CACHE = {}


def _build_program():
    nc = bacc.Bacc("TRN2", target_bir_lowering=False, debug=False, num_devices=NCORES)

    xaugT = nc.dram_tensor("xaugT", [KIN, NLOC], F32, kind="ExternalInput").ap()
    w1augT = nc.dram_tensor("w1augT", [KIN, MID], F32, kind="ExternalInput").ap()
    w2sb16 = nc.dram_tensor("w2sb16", [MID, BOT], F16, kind="ExternalInput").ap()
    midvec = nc.dram_tensor("midvec", [MID, 2], F32, kind="ExternalInput").ap()
    t2c = nc.dram_tensor("t2c", [128, MC], F32, kind="ExternalInput").ap()
    identh = nc.dram_tensor("identh", [128, 128], F16, kind="ExternalInput").ap()
    out_d = nc.dram_tensor("out", [NLOC, BOT], F32, kind="ExternalOutput").ap()

    with tile.TileContext(nc) as tc, \
         tc.tile_pool(name="const", bufs=1) as cpool, \
         tc.tile_pool(name="ab", bufs=1) as abpool, \
         tc.tile_pool(name="mx", bufs=3) as mxpool, \
         tc.tile_pool(name="y2", bufs=3) as ypool, \
         tc.tile_pool(name="work", bufs=2) as wpool, \
         tc.tile_pool(name="mm", bufs=3, space="PSUM") as mmpool, \
         tc.tile_pool(name="dps", bufs=2, space="PSUM") as dpool:

        # ---- load constants (spread across SEQs: each dma_start costs
        # ~600ns of issuing-sequencer time, so one queue would serialize) ----
        xaug_sb = cpool.tile([KIN, NLOC], F32R)
        w1aug_sb = cpool.tile([KIN, MID], F32R)
        idh_sb = cpool.tile([128, 128], F16)
        t2_sb = cpool.tile([128, MC], F32)
        # w1aug chunk 0 first (tiny) so the first phase-0 matmul can start
        # while the rest of the weights stream in
        nc.scalar.dma_start(w1aug_sb[:, 0:128], w1augT[:, 0:128].bitcast(F32R))
        nc.sync.dma_start(xaug_sb[:], xaugT.bitcast(F32R))
        nc.scalar.dma_start(w1aug_sb[:, 128:MID], w1augT[:, 128:MID].bitcast(F32R))
        # posT / w1b are row slices of tensors already in SBUF
        posT_sb = xaug_sb
        w1b_sb = w1aug_sb
        # mv chunks first on the SWDGE queue: phase-0 activations need them
        # almost immediately; idh/t2 aren't read until ~10us in
        w2b_sb, mv_sb = [], []
        for k in range(KC):
            mv = cpool.tile([128, 2], F32, tag=f"mv{k}")
            nc.gpsimd.dma_start(mv[:], midvec[k * 128:(k + 1) * 128, :])
            mv_sb.append(mv)
        for k in range(KC):
            wb = cpool.tile([128, BOT], F16, tag=f"w2b{k}")
            (nc.sync if k % 2 else nc.scalar).dma_start(
                wb[:], w2sb16[k * 128:(k + 1) * 128, :])
            w2b_sb.append(wb)
        nc.gpsimd.dma_start(t2_sb[:], t2c)
        nc.gpsimd.dma_start(idh_sb[:], identh)

        # ---- phase 0: A'' on DVE (tensor_scalar affine), dup-B on ACT.
        # B'' is read as a stride-2 view of B_dup, so ACT's phase-0 chain is
        # one pass per chunk (it gates how early DVE can start scene-0 Mx) ----
        A_bf, B_bf, B_dup = [], [], []
        for k in range(KC):
            ck = slice(k * 128, (k + 1) * 128)
            ps = mmpool.tile([128, 1024], F32, tag="ps")
            nc.tensor.matmul(ps[:, 0:512], lhsT=w1aug_sb[:, ck], rhs=xaug_sb[:],
                             start=True, stop=True)
            nc.tensor.matmul(ps[:, 512:1024], lhsT=w1b_sb[H:H + 2, ck], rhs=posT_sb[H:H + 2, :],
                             start=True, stop=True)
            a_t = abpool.tile([128, NLOC], F16, tag=f"A{k}")
            nc.scalar.activation(
                a_t[:], ps[:, 0:512], mybir.ActivationFunctionType.Identity,
                bias=mv_sb[k][:, 1:2], scale=mv_sb[k][:, 0:1])
            A_bf.append(a_t)
            # duplicated fp16 copy: B_dup[c, 2n+q] = B''[c, n], q in {0,1}
            bd = abpool.tile([128, 2 * NLOC], F16, tag=f"Bd{k}")
            nc.scalar.activation(
                bd[:].rearrange("c (n q) -> c n q", q=2),
                ps[:, 512:1024].unsqueeze(2).broadcast_to((128, NLOC, 2)),
                mybir.ActivationFunctionType.Identity,
                bias=0.0, scale=mv_sb[k][:, 0:1])
            B_dup.append(bd)
            B_bf.append(bd[:].rearrange("c (n q) -> c n q", q=2))

        # ---- all D terms upfront (PE filler while DVE builds scene-0 Mx):
        # d16[c, (s, m, i)] = (W2s chunk-m)^T @ B''_scene, drained to fp16 ----
        d16 = abpool.tile([128, S_LOC * MC * P], F16, tag="d16")
        for s in range(S_LOC):
            for m in range(MC):
                dp = dpool.tile([128, P], F32, tag="dps", name="dp")
                for k in range(KC):
                    nc.tensor.matmul(dp[:], lhsT=w2b_sb[k][:, m * 128:(m + 1) * 128],
                                     rhs=B_bf[k][:, s * P:(s + 1) * P, 0],
                                     start=(k == 0), stop=(k == KC - 1))
                idx = s * MC + m
                nc.scalar.copy(d16[:, idx * P:(idx + 1) * P], dp[:])

        def epilogue(sp, maxraw_p):
            # epilogue in [m-chunk, i] orientation:
            #   o_T = maxraw - d16 (DVE fp16 at 2x),
            #   fp16 PE transpose, ACT relu (fp16 PSUM -> f32 SBUF), DMA out.
            # The last scene DMAs each m-half as soon as its relu lands so
            # the final tree only gates half the output.
            last = sp == S_LOC - 1
            o_sb = wpool.tile([P, BOT], F32, tag="osb")
            for m in range(MC):
                idx = sp * MC + m
                o_t = wpool.tile([128, P], F16, tag=f"ot{m}", name=f"ot_{m}")
                nc.vector.tensor_tensor(
                    out=o_t[:], in0=maxraw_p[m][:],
                    in1=d16[:, idx * P:(idx + 1) * P],
                    op=mybir.AluOpType.subtract)
                # transpose PSUM comes from the dps pool (same bank size,
                # only used during the upfront D phase) via a bitcast view
                tr = dpool.tile([128, P], F32, tag="dps", name="tr")
                trv = tr[:].bitcast(F16)[0:P, :]
                nc.tensor.transpose(trv, o_t[:], idh_sb[:])
                nc.scalar.activation(o_sb[:, m * 128:(m + 1) * 128], trv,
                                     mybir.ActivationFunctionType.Relu)
                if last:
                    nc.sync.dma_start(
                        out_d[sp * P:(sp + 1) * P, m * 128:(m + 1) * 128],
                        o_sb[:, m * 128:(m + 1) * 128])
            if not last:
                nc.sync.dma_start(out_d[sp * P:(sp + 1) * P, :], o_sb[:])

        # ---- per-scene pipeline (epilogue of scene s-1 is emitted inside
        # scene s so no engine stream waits on same-scene late results) ----
        def emit_mx(s):
            # Mx[c, i, j] = max(A''[c, j], B''[c, i]) in fp16 at 2x.
            # Layout [c, (i, jw, q)] with j = 2*jw + q; every operand's
            # innermost AP dim is [step 1, 2 elems] so DVE picks 2x_1P.
            cs = slice(s * P, (s + 1) * P)
            mx = [mxpool.tile([128, P * P], F16, tag=f"mx{k}", name=f"mx_{k}")
                  for k in range(KC)]

            def mx_tt(k, ilo, ihi):
                ni = ihi - ilo
                nc.vector.tensor_tensor(
                    out=mx[k][:, ilo * P:ihi * P]
                        .rearrange("c (i w q) -> c i w q", w=P // 2, q=2),
                    in0=A_bf[k][:, cs].rearrange("c (w q) -> c w q", q=2)
                        .unsqueeze(1).broadcast_to((128, ni, P // 2, 2)),
                    in1=B_dup[k][:, 2 * (s * P + ilo):2 * (s * P + ihi)]
                        .rearrange("c (i q) -> c i q", q=2)
                        .unsqueeze(2).broadcast_to((128, ni, P // 2, 2)),
                    op=mybir.AluOpType.max)

            if s == 0:
                # Scene 0 in i-quarters so the first L2 block-pair can start
                # after ~1/4 of the Mx work; quarter 0 reads B'' directly
                # (1x) so it isn't gated on the B_dup ACT chain.
                for k in range(KC):
                    nc.vector.tensor_tensor(
                        out=mx[k][:, 0:16 * P]
                            .rearrange("c (i w q) -> c i w q", w=P // 2, q=2),
                        in0=A_bf[k][:, cs].rearrange("c (w q) -> c w q", q=2)
                            .unsqueeze(1).broadcast_to((128, 16, P // 2, 2)),
                        in1=B_bf[k][:, s * P:s * P + 16, 0]
                            .unsqueeze(2).unsqueeze(3)
                            .broadcast_to((128, 16, P // 2, 2)),
                        op=mybir.AluOpType.max)
                for quarter in range(1, 4):
                    for k in range(KC):
                        mx_tt(k, quarter * 16, (quarter + 1) * 16)
            else:
                for k in range(KC):
                    mx_tt(k, 0, P)
            return mx

        prev = None
        mx_next = None
        for s in range(S_LOC):
            mx = mx_next if mx_next is not None else emit_mx(s)
            # hoist the last scene's Mx ahead of scene-6's drain-dependent
            # trees so DVE never delays the final L2 stream
            mx_next = emit_mx(s + 1) if s == S_LOC - 2 else None

            maxraw = [wpool.tile([128, P], F16, tag=f"mr{m}", name=f"mr_{m}")
                      for m in range(MC)]
            for m in range(MC):
                fine_tail = (s == S_LOC - 1 and m == MC - 1)
                # layer-2 matmuls (fp16); ACT drains 1024-wide with bias t2
                y2 = ypool.tile([128, P * P], F16, tag=f"y2{m}", name=f"y2_{m}")
                y3 = y2[:].rearrange("c (i j) -> c i j", j=P)
                r1 = wpool.tile([128, P * 32], F16, tag="r1", name="r1")
                for bp in range(4):
                    ps_t = mmpool.tile([128, 1024], F32, tag="ps")
                    for half in range(2):
                        bs = slice((2 * bp + half) * 512, (2 * bp + half + 1) * 512)
                        for k in range(KC):
                            nc.tensor.matmul(
                                ps_t[:, half * 512:(half + 1) * 512],
                                lhsT=w2b_sb[k][:, m * 128:(m + 1) * 128],
                                rhs=mx[k][:, bs],
                                start=(k == 0), stop=(k == KC - 1))
                    nc.scalar.activation(
                        y2[:, bp * 1024:(bp + 1) * 1024], ps_t[:],
                        mybir.ActivationFunctionType.Identity,
                        bias=t2_sb[:, m:m + 1], scale=1.0)
                    if fine_tail:
                        # last scene: fully fine-grained trees, interleaved
                        # with the drains, to shorten the pipeline tail
                        isl = slice(bp * 16, (bp + 1) * 16)
                        r1v = r1[:].rearrange("c (i j) -> c i j", j=32)
                        nc.vector.tensor_tensor(
                            out=r1v[:, isl, :],
                            in0=y3[:, isl, 0:32], in1=y3[:, isl, 32:64],
                            op=mybir.AluOpType.max)
                        r2f = wpool.tile([128, P * 16], F16, tag="r2", name="r2f")
                        r2fv = r2f[:].rearrange("c (i j) -> c i j", j=16)
                        nc.vector.tensor_tensor(
                            out=r2fv[:, isl, :],
                            in0=r1v[:, isl, 0:16], in1=r1v[:, isl, 16:32],
                            op=mybir.AluOpType.max)
                        r3f = wpool.tile([128, P * 8], F16, tag="r3", name="r3f")
                        r3fv = r3f[:].rearrange("c (i j) -> c i j", j=8)
                        nc.vector.tensor_tensor(
                            out=r3fv[:, isl, :],
                            in0=r2fv[:, isl, 0:8], in1=r2fv[:, isl, 8:16],
                            op=mybir.AluOpType.max)
                        nc.vector.tensor_reduce(
                            out=maxraw[m][:, isl],
                            in_=r3fv[:, isl, :],
                            axis=mybir.AxisListType.X,
                            op=mybir.AluOpType.max)

                # j-max: TT-max halvings at 2x all the way down (the 1x
                # grouped reduce is slower than three tiny 2x TTs)
                if not fine_tail:
                    nc.vector.tensor_tensor(
                        out=r1[:].rearrange("c (i j) -> c i j", j=32),
                        in0=y3[:, :, 0:32], in1=y3[:, :, 32:64],
                        op=mybir.AluOpType.max)
                    r1v = r1[:].rearrange("c (i j) -> c i j", j=32)
                    r2 = wpool.tile([128, P * 16], F16, tag="r2", name="r2")
                    nc.vector.tensor_tensor(
                        out=r2[:].rearrange("c (i j) -> c i j", j=16),
                        in0=r1v[:, :, 0:16], in1=r1v[:, :, 16:32],
                        op=mybir.AluOpType.max)
                    r2v = r2[:].rearrange("c (i j) -> c i j", j=16)
                    r3 = wpool.tile([128, P * 8], F16, tag="r3", name="r3")
                    nc.vector.tensor_tensor(
                        out=r3[:].rearrange("c (i j) -> c i j", j=8),
                        in0=r2v[:, :, 0:8], in1=r2v[:, :, 8:16],
                        op=mybir.AluOpType.max)
                    r3v = r3[:].rearrange("c (i j) -> c i j", j=8)
                    r4 = wpool.tile([128, P * 4], F16, tag="r4", name="r4")
                    nc.vector.tensor_tensor(
                        out=r4[:].rearrange("c (i j) -> c i j", j=4),
                        in0=r3v[:, :, 0:4], in1=r3v[:, :, 4:8],
                        op=mybir.AluOpType.max)
                    r4v = r4[:].rearrange("c (i j) -> c i j", j=4)
                    r5 = wpool.tile([128, P * 2], F16, tag="r5", name="r5")
                    nc.vector.tensor_tensor(
                        out=r5[:].rearrange("c (i j) -> c i j", j=2),
                        in0=r4v[:, :, 0:2], in1=r4v[:, :, 2:4],
                        op=mybir.AluOpType.max)
                    r5v = r5[:].rearrange("c (i j) -> c i j", j=2)
                    nc.vector.tensor_tensor(
                        out=maxraw[m][:].unsqueeze(2),
                        in0=r5v[:, :, 0:1], in1=r5v[:, :, 1:2],
                        op=mybir.AluOpType.max)

            # previous scene's epilogue (keeps PE/DVE streams off
            # same-scene late results)
            if prev is not None:
                epilogue(*prev)
            prev = (s, maxraw)
        epilogue(*prev)

    nc.compile()
    return nc


def _prep_inputs(inputs):
    h = np.ascontiguousarray(inputs["h_states"], np.float32)
    pos = np.ascontiguousarray(inputs["last_pos"], np.float32)
    spd = np.ascontiguousarray(inputs["speed"], np.float32)
    W_emb = np.asarray(inputs["W_emb"], np.float32)
    b_emb = np.asarray(inputs["b_emb"], np.float32)
    W1 = np.asarray(inputs["W1"], np.float32)
    b1 = np.asarray(inputs["b1"], np.float32)
    g1 = np.asarray(inputs["g1"], np.float32)
    be1 = np.asarray(inputs["be1"], np.float32)
    m1 = np.asarray(inputs["m1"], np.float32)
    v1 = np.asarray(inputs["v1"], np.float32)
    W2 = np.asarray(inputs["W2"], np.float32)
    b2 = np.asarray(inputs["b2"], np.float32)
    g2 = np.asarray(inputs["g2"], np.float32)
    be2 = np.asarray(inputs["be2"], np.float32)
    m2 = np.asarray(inputs["m2"], np.float32)
    v2 = np.asarray(inputs["v2"], np.float32)

    s1 = g1 / np.sqrt(v1 + EPS)
    t1 = be1 - m1 * s1
    s2 = g2 / np.sqrt(v2 + EPS)
    t2 = be2 - m2 * s2 + b2 * s2
    R3 = W_emb @ W1[H:H + E, :]                       # [3, MID]
    W1aug = np.concatenate([W1[:H, :], R3], axis=0)   # [67, MID]
    c0v = b1 + b_emb @ W1[H:H + E, :]                 # [MID]
    ca = s1 * c0v + t1
    W2s = W2 * s2[None, :]                            # [MID, BOT]

    X = np.concatenate([h, pos[:, 0:1], pos[:, 1:2], spd], axis=1)  # [N, 67]

    common = dict(
        w1augT=np.ascontiguousarray(W1aug, np.float32),
        w2sb16=np.ascontiguousarray(W2s.astype(np.float16)),
        midvec=np.ascontiguousarray(np.stack([s1, ca], axis=1), np.float32),
        t2c=np.ascontiguousarray(t2.reshape(MC, 128).T, np.float32),
        identh=np.eye(128, dtype=np.float16),
    )
    in_maps = []
    for c in range(NCORES):
        xc = X[c * NLOC:(c + 1) * NLOC, :]            # [512, 67]
        m = dict(common)
        m["xaugT"] = np.ascontiguousarray(xc.T, np.float32)
        in_maps.append(m)
    return in_maps


def kernel(**inputs):
    if "nc" not in _CACHE:
        _CACHE["nc"] = _build_program()
    nc = _CACHE["nc"]
    in_maps = _prep_inputs(inputs)
    res = run_bass_kernel_spmd(nc, in_maps, list(range(NCORES)))
    out = np.concatenate([res.results[c]["out"] for c in range(NCORES)], axis=0)
    return np.ascontiguousarray(out, np.float32)
